# revision 13
# baseline (speedup 1.0000x reference)
"""Trainium2 Bass kernel for nn_Decoder (LSTM + dot-attention + vocab projection).

Distribution (8 NeuronCores, single SPMD launch):
- fp16 hi/lo x3-term matmul splits everywhere -> fp32-identical numerics at
  1 cyc/row on the PE (plain fp32 is 4 cyc/row; plain bf16/f32r flip argmaxes).
- P1: XG = x @ W_ih.T (+gate biases), batch-sharded (4 rows/core) + AllGather.
- P2: po = output @ attn_w, po2 = output @ Wc2.T for the core's 4 batch rows.
- P3: LSTM recurrence replicated on all cores (teacher forcing makes attention
  independent of the recurrence), transposed layout [H-part, batch-free].
- P4: attention + yc for local batch rows, batched over all 39 steps; second
  AllGather shares yc.
- P5: vocab projection V-sharded (3750 rows/core) with on-device top-1.
Host does input marshalling (embedding gather, transposes, fp16 splits,
shards) and output unsharding (concat + 8-way argmax combine).
"""

import numpy as np

B, L, T, H, V = 32, 80, 40, 512, 30000
SOS = 1
S = T - 1       # 39 decode steps needed
NC = 8
VS = V // NC    # 3750
BLOC = B // NC  # 4
BT = B * T      # 1280 projection rows (index b*40+s; s=0 rows are SOS)
NLOC = BLOC * S  # 156

_CACHE = {}


def _split16(x):
    hi = x.astype(np.float16)
    lo = (x.astype(np.float32) - hi.astype(np.float32)).astype(np.float16)
    return hi, lo


def _build_program():
    import concourse.bass as bass
    import concourse.mybir as mybir
    import concourse.tile as tile
    from concourse import bacc
    from concourse.masks import make_identity

    f32 = mybir.dt.float32
    f16 = mybir.dt.float16
    u32 = mybir.dt.uint32
    AF = mybir.ActivationFunctionType
    ALU = mybir.AluOpType
    AX = mybir.AxisListType

    nc = bacc.Bacc("TRN2", target_bir_lowering=False, debug=False, num_devices=NC)

    def din(name, shape, dt=f32):
        return nc.dram_tensor(name, shape, dt, kind="ExternalInput").ap()

    def dout(name, shape, dt=f32):
        return nc.dram_tensor(name, shape, dt, kind="ExternalOutput").ap()

    whh_hi = din("whh_hi", [512, 2048], f16)
    whh_lo = din("whh_lo", [512, 2048], f16)
    wih_hi = din("wih_hi", [512, 2048], f16)
    wih_lo = din("wih_lo", [512, 2048], f16)
    attn_hi = din("attn_hi", [512, 513], f16)   # attn_w | attn_b col
    attn_lo = din("attn_lo", [512, 513], f16)
    wc1t_hi = din("wc1t_hi", [512, 512], f16)
    wc1t_lo = din("wc1t_lo", [512, 512], f16)
    wc2t_hi = din("wc2t_hi", [512, 512], f16)
    wc2t_lo = din("wc2t_lo", [512, 512], f16)
    wout_hi = din("wout_hi", [512, VS], f16)    # per-core V-shard, transposed
    wout_lo = din("wout_lo", [512, VS], f16)
    xt_hi = din("xt_hi", [128, 4, NLOC], f16)   # per-core embed(x).T slice
    xt_lo = din("xt_lo", [128, 4, NLOC], f16)
    sos_hi = din("sos_hi", [128, 4], f16)
    sos_lo = din("sos_lo", [128, 4], f16)
    out_loc = din("out_loc", [BLOC * L, 512])
    h0t = din("h0t", [128, 4, 32])
    c0t = din("c0t", [128, 4, 32])
    bsum = din("bsum", [1, 2048])
    cbias = din("cbias", [128, 4])
    rank_oh = din("rank_oh", [128, NC])

    dec = dout("dec", [1249, VS])
    mxo = dout("mxo", [128, 10])
    mio = dout("mio", [128, 10], u32)

    def rkp(ap, expr="(k p) x -> p k x"):
        return ap.rearrange(expr, p=128)

    with tile.TileContext(nc) as tc:
        with tc.tile_pool(name="per", bufs=1) as per, tc.tile_pool(
            name="dramp", bufs=1, space="DRAM"
        ) as dramp:
            # ---------- persistent loads (live through P5) ----------
            woutH = per.tile([128, 4, VS], f16, name="woutH")
            nc.sync.dma_start(woutH, rkp(wout_hi))
            woutL = per.tile([128, 4, VS], f16, name="woutL")
            nc.sync.dma_start(woutL, rkp(wout_lo))
            sosH = per.tile([128, 4], f16, name="sosH")
            nc.sync.dma_start(sosH, sos_hi)
            sosL = per.tile([128, 4], f16, name="sosL")
            nc.sync.dma_start(sosL, sos_lo)

            # xg layouts: shard [s, p, g*b] so per-step reads are contiguous
            xg_shard = dramp.tile([S, 128, 64], f32, name="xg_shard")
            xg_full = dramp.tile(
                [NC, S, 128, 64], f32, name="xg_full", addr_space="Shared"
            )
            yct_shard = dramp.tile([2, 512, NLOC], f16, name="yct_shard")
            yct_full = dramp.tile(
                [NC, 2, 512, NLOC], f16, name="yct_full", addr_space="Shared"
            )

            mid = tc.alloc_tile_pool(name="mid", bufs=1)
            whhH = mid.tile([128, 4, 2048], f16, name="whhH")
            nc.sync.dma_start(whhH, rkp(whh_hi))
            whhL = mid.tile([128, 4, 2048], f16, name="whhL")
            nc.sync.dma_start(whhL, rkp(whh_lo))
            attnH = mid.tile([128, 4, 513], f16, name="attnH")
            nc.sync.dma_start(attnH, rkp(attn_hi))
            attnL = mid.tile([128, 4, 513], f16, name="attnL")
            nc.sync.dma_start(attnL, rkp(attn_lo))
            wc1H = mid.tile([128, 4, 512], f16, name="wc1H")
            nc.sync.dma_start(wc1H, rkp(wc1t_hi))
            wc1L = mid.tile([128, 4, 512], f16, name="wc1L")
            nc.sync.dma_start(wc1L, rkp(wc1t_lo))
            wc2H = mid.tile([128, 4, 512], f16, name="wc2H")
            nc.sync.dma_start(wc2H, rkp(wc2t_hi))
            wc2L = mid.tile([128, 4, 512], f16, name="wc2L")
            nc.sync.dma_start(wc2L, rkp(wc2t_lo))
            cb_sb = mid.tile([128, 4], f32, name="cb_sb")
            nc.sync.dma_start(cb_sb, cbias)
            rank_sb = mid.tile([128, NC], f32, name="rank_sb")
            nc.sync.dma_start(rank_sb, rank_oh)
            ones39 = mid.tile([1, S], f32, name="ones39")
            nc.vector.memset(ones39, 1.0)
            ident = mid.tile([128, 128], f32, name="ident")
            make_identity(nc, ident[:])

            # state: HT16[p, k, th, 0:32]=h_hi, [32:64]=h_lo; slot 39 holds h0
            HT16 = mid.tile([128, 4, 40, 64], f16, name="HT16")
            cT = mid.tile([128, 4, 32], f32, name="cT")
            nc.sync.dma_start(cT, c0t)
            h32i = mid.tile([128, 4, 32], f32, name="h32i")
            nc.sync.dma_start(h32i, h0t)
            nc.vector.tensor_copy(HT16[:, :, 39, 0:32], h32i[:])
            nc.vector.tensor_tensor(
                out=HT16[:, :, 39, 32:64], in0=h32i[:], in1=HT16[:, :, 39, 0:32],
                op=ALU.subtract,
            )

            # ---------- P1: XG shard + AllGather ----------
            sc1 = nc.enter_named_scope("P1_xg", False)
            with tc.tile_pool(name="p1", bufs=1) as p1, tc.tile_pool(
                name="p1ps", bufs=1, space="PSUM"
            ) as p1ps:
                wihH = p1.tile([128, 4, 2048], f16, name="wihH")
                nc.sync.dma_start(wihH, rkp(wih_hi))
                wihL = p1.tile([128, 4, 2048], f16, name="wihL")
                nc.sync.dma_start(wihL, rkp(wih_lo))
                xtH = p1.tile([128, 4, NLOC], f16, name="xtH")
                nc.sync.dma_start(xtH, xt_hi)
                xtL = p1.tile([128, 4, NLOC], f16, name="xtL")
                nc.sync.dma_start(xtL, xt_lo)
                bsum_sb = p1.tile([1, 2048], f32, name="bsum_sb")
                nc.sync.dma_start(bsum_sb, bsum)
                ones156 = p1.tile([1, NLOC], f32, name="ones156")
                nc.vector.memset(ones156, 1.0)
                xg_stage2 = p1.tile([128, S, 16, BLOC], f32, name="xg_stage2")
                for g in range(16):
                    ps = p1ps.tile([128, NLOC], f32, name=f"psxg{g}", tag="psxg", bufs=4)
                    gs = slice(g * 128, (g + 1) * 128)
                    nc.tensor.matmul(ps[:], bsum_sb[:, gs], ones156[:], start=True, stop=False)
                    for k in range(4):
                        nc.tensor.matmul(ps[:], wihH[:, k, gs], xtH[:, k, :], start=False, stop=False)
                    for k in range(4):
                        nc.tensor.matmul(ps[:], wihH[:, k, gs], xtL[:, k, :], start=False, stop=False)
                    for k in range(4):
                        nc.tensor.matmul(ps[:], wihL[:, k, gs], xtH[:, k, :], start=False, stop=(k == 3))
                    nc.scalar.copy(
                        xg_stage2[:, :, g, :], ps[:].rearrange("p (b s) -> p s b", b=BLOC)
                    )
                nc.sync.dma_start(
                    xg_shard.rearrange("s p gb -> p s gb"), 
                    xg_stage2[:].rearrange("p s g b -> p s (g b)"),
                )
            nc.gpsimd.collective_compute(
                "AllGather", ALU.bypass, replica_groups=[list(range(NC))],
                ins=[xg_shard.opt()], outs=[xg_full.opt()],
            )
            nc.leave_named_scope("P1_xg", sc1[0], False)
            sc2 = nc.enter_named_scope("P2_po", False)

            # ---------- P2: outT, po (+s0), po2 ----------
            poH = mid.tile([128, 4, 4, 80], f16, name="poH")
            poL = mid.tile([128, 4, 4, 80], f16, name="poL")
            s0_sb = mid.tile([1, 4, 80], f32, name="s0_sb")
            po2 = mid.tile([80, 4, 512], f32, name="po2")
            with tc.tile_pool(name="p2", bufs=1) as p2, tc.tile_pool(
                name="p2ps", bufs=1, space="PSUM"
            ) as p2ps:
                oin = p2.tile([128, 3, 512], f32, name="oin")
                for rt, rsz in ((0, 128), (1, 128), (2, 64)):
                    nc.sync.dma_start(
                        oin[0:rsz, rt, :], out_loc[rt * 128:rt * 128 + rsz, :]
                    )
                ot32 = p2.tile([128, 4, 320], f32, name="ot32")
                for rt, rsz in ((0, 128), (1, 128), (2, 64)):
                    for hk in range(4):
                        pst = p2ps.tile([128, 128], f32, name=f"pst{rt}_{hk}", tag="pst", bufs=2)
                        nc.tensor.transpose(
                            pst[:, 0:rsz],
                            oin[0:rsz, rt, hk * 128:(hk + 1) * 128],
                            ident[0:rsz, 0:rsz],
                        )
                        nc.scalar.copy(ot32[:, hk, rt * 128:rt * 128 + rsz], pst[:, 0:rsz])
                otH = p2.tile([128, 4, 320], f16, name="otH")
                otL = p2.tile([128, 4, 320], f16, name="otL")
                nc.vector.tensor_copy(otH[:], ot32[:])
                nc.vector.tensor_tensor(out=otL[:], in0=ot32[:], in1=otH[:], op=ALU.subtract)
                for m in range(5):  # 4 kout tiles + s0 row
                    msl = slice(m * 128, (m + 1) * 128) if m < 4 else slice(512, 513)
                    mp = 128 if m < 4 else 1
                    ps = p2ps.tile([128, 320], f32, name=f"pspo{m}", tag="pspo", bufs=2)
                    for k in range(4):
                        nc.tensor.matmul(ps[0:mp, :], attnH[:, k, msl], otH[:, k, :], start=(k == 0), stop=False)
                    for k in range(4):
                        nc.tensor.matmul(ps[0:mp, :], attnH[:, k, msl], otL[:, k, :], start=False, stop=False)
                    for k in range(4):
                        nc.tensor.matmul(ps[0:mp, :], attnL[:, k, msl], otH[:, k, :], start=False, stop=(k == 3))
                    if m < 4:
                        nc.vector.tensor_copy(
                            poH[:, m, :, :], ps[:].rearrange("p (b l) -> p b l", b=4)
                        )
                        nc.vector.tensor_tensor(
                            out=poL[:, m, :, :],
                            in0=ps[:].rearrange("p (b l) -> p b l", b=4),
                            in1=poH[:, m, :, :], op=ALU.subtract,
                        )
                    else:
                        nc.vector.tensor_copy(
                            s0_sb[:], ps[0:1, :].rearrange("p (b l) -> p b l", b=4)
                        )
                for b in range(4):
                    ps = p2ps.tile([80, 512], f32, name=f"pspo2_{b}", tag="pspo2", bufs=2)
                    bsl = slice(b * 80, b * 80 + 80)
                    for k in range(4):
                        nc.tensor.matmul(ps[:], otH[:, k, bsl], wc2H[:, k, :], start=(k == 0), stop=False)
                    for k in range(4):
                        nc.tensor.matmul(ps[:], otL[:, k, bsl], wc2H[:, k, :], start=False, stop=False)
                    for k in range(4):
                        nc.tensor.matmul(ps[:], otH[:, k, bsl], wc2L[:, k, :], start=False, stop=(k == 3))
                    nc.scalar.copy(po2[:, b, :], ps[:])

            nc.leave_named_scope("P2_po", sc2[0], False)
            sc3 = nc.enter_named_scope("P3_rec", False)

            # ---------- P3: LSTM recurrence ----------
            def xg_step_src(t):
                # [p][c][g*b] with 256B-contiguous per-partition runs
                return bass.AP(
                    tensor=xg_full.tensor,
                    offset=xg_full.offset + t * 128 * 64,
                    ap=[[64, 128], [S * 128 * 64, NC], [1, 64]],
                )

            with tc.tile_pool(name="p3", bufs=1) as p3, tc.tile_pool(
                name="p3ps", bufs=1, space="PSUM"
            ) as p3ps:
                for t in range(S):
                    xgt = p3.tile([128, NC, 16, BLOC], f32, name=f"xgt{t}", tag="xgt", bufs=3)
                    nc.sync.dma_start(xgt[:], xg_step_src(t))
                    psg = p3ps.tile([128, 4, 4, 2, 32], f32, name=f"psg{t}", tag="psg", bufs=2)
                    # psg dims: [p, grp(i,f,o,g~), hc, {hi,lo}, b]
                    th = 39 if t == 0 else t - 1
                    xgt2 = p3.tile([128, 16, NC, BLOC], f32, name=f"xgt2_{t}", tag="xgt2", bufs=2)
                    nc.vector.tensor_copy(xgt2[:], xgt[:].transpose([0, 2, 1, 3]))
                    xgv = xgt2[:].rearrange("p (grp hc) c b -> p grp hc c b", hc=4)
                    for half in range(2):
                        hsl = slice(2 * half, 2 * half + 2)
                        # PE: all gtiles of this half; k-pairs split so next
                        # step can start on half-produced h chunks
                        for grp in range(4):
                            for hc in (2 * half, 2 * half + 1):
                                g = grp * 4 + hc
                                gs = slice(g * 128, (g + 1) * 128)
                                out_w = psg[:, grp, hc, :, :]
                                out_h = psg[:, grp, hc, 0, :]
                                for kk in range(2):
                                    for k in (2 * kk, 2 * kk + 1):
                                        nc.tensor.matmul(
                                            out_w, whhH[:, k, gs], HT16[:, k, th, :],
                                            start=(k == 0), stop=False,
                                        )
                                    for k in (2 * kk, 2 * kk + 1):
                                        nc.tensor.matmul(
                                            out_h, whhL[:, k, gs], HT16[:, k, th, 0:32],
                                            start=False, stop=(k == 3),
                                            skip_group_check=True,
                                        )
                        gsb = p3.tile([128, 4, 2, 32], f32, name=f"gsb{t}_{half}", tag=f"gsb{half}", bufs=2)
                        nc.vector.tensor_tensor(
                            out=gsb[:].rearrange("p grp hc (c b) -> p grp hc c b", c=NC),
                            in0=xgv[:, :, hsl, :, :],
                            in1=psg[:, :, hsl, 0, :].rearrange("p grp hc (c b) -> p grp hc c b", c=NC),
                            op=ALU.add,
                        )
                        nc.vector.tensor_tensor(
                            out=gsb[:], in0=gsb[:], in1=psg[:, :, hsl, 1, :], op=ALU.add,
                        )
                        asb = p3.tile([128, 3, 2, 32], f32, name=f"asb{t}_{half}", tag=f"asb{half}", bufs=2)
                        nc.scalar.activation(asb[:], gsb[:, 0:3, :, :], AF.Sigmoid)
                        tnh = p3.tile([128, 2, 32], f32, name=f"tnh{t}_{half}", tag=f"tnh{half}", bufs=2)
                        nc.scalar.activation(tnh[:], gsb[:, 3, :, :], AF.Tanh)
                        t1 = p3.tile([128, 2, 32], f32, name=f"t1_{t}_{half}", tag=f"t1{half}", bufs=2)
                        nc.vector.tensor_tensor(out=t1[:], in0=asb[:, 0, :, :], in1=tnh[:], op=ALU.mult)
                        t2 = p3.tile([128, 2, 32], f32, name=f"t2_{t}_{half}", tag=f"t2{half}", bufs=2)
                        nc.vector.tensor_tensor(out=t2[:], in0=asb[:, 1, :, :], in1=cT[:, hsl, :], op=ALU.mult)
                        nc.vector.tensor_tensor(out=cT[:, hsl, :], in0=t1[:], in1=t2[:], op=ALU.add)
                        tc32 = p3.tile([128, 2, 32], f32, name=f"tc32_{t}_{half}", tag=f"tc32{half}", bufs=2)
                        nc.scalar.activation(tc32[:], cT[:, hsl, :], AF.Tanh)
                        h32 = p3.tile([128, 2, 32], f32, name=f"h32_{t}_{half}", tag=f"h32{half}", bufs=2)
                        nc.vector.tensor_tensor(out=h32[:], in0=asb[:, 2, :, :], in1=tc32[:], op=ALU.mult)
                        nc.scalar.copy(HT16[:, hsl, t, 0:32], h32[:])
                        nc.vector.tensor_tensor(
                            out=HT16[:, hsl, t, 32:64], in0=h32[:],
                            in1=HT16[:, hsl, t, 0:32], op=ALU.subtract,
                        )
            nc.leave_named_scope("P3_rec", sc3[0], False)
            sc4 = nc.enter_named_scope("P4_attn", False)

            # ---------- P4: attention + yc (local 4 b) ----------
            with tc.tile_pool(name="p4", bufs=1) as p4, tc.tile_pool(
                name="p4ps", bufs=1, space="PSUM"
            ) as p4ps:
                # local-h extraction via rank one-hot: hsel32 = sum_c r[c]*HT16[..4c+b..]
                hsel32 = p4.tile([128, 4, S, 2, 4], f32, name="hsel32")
                tmp = p4.tile([128, 4, S, 2, 4], f32, name="tmpsel")
                for c in range(NC):
                    dst = tmp if c > 0 else hsel32
                    nc.vector.tensor_scalar(
                        out=dst[:, :, :, 0, :], in0=HT16[:, :, 0:S, 4 * c:4 * c + 4],
                        scalar1=rank_sb[:, c:c + 1], scalar2=None, op0=ALU.mult,
                    )
                    nc.vector.tensor_scalar(
                        out=dst[:, :, :, 1, :],
                        in0=HT16[:, :, 0:S, 32 + 4 * c:32 + 4 * c + 4],
                        scalar1=rank_sb[:, c:c + 1], scalar2=None, op0=ALU.mult,
                    )
                    if c > 0:
                        nc.vector.tensor_tensor(out=hsel32[:], in0=hsel32[:], in1=tmp[:], op=ALU.add)
                hsel = p4.tile([128, 4, S, 2, 4], f16, name="hsel")
                nc.vector.tensor_copy(hsel[:], hsel32[:])

                ps_sc = p4ps.tile([S, 4, 80], f32, name="ps_sc")
                for b in range(4):
                    for k in range(4):
                        nc.tensor.matmul(
                            ps_sc[:, b, :], hsel[:, k, :, 0, b], poH[:, k, b, :],
                            start=(k == 0), stop=False,
                        )
                    for k in range(4):
                        nc.tensor.matmul(
                            ps_sc[:, b, :], hsel[:, k, :, 1, b], poH[:, k, b, :],
                            start=False, stop=False,
                        )
                    for k in range(4):
                        nc.tensor.matmul(
                            ps_sc[:, b, :], hsel[:, k, :, 0, b], poL[:, k, b, :],
                            start=False, stop=False,
                        )
                    nc.tensor.matmul(
                        ps_sc[:, b, :], ones39[:], s0_sb[:, b, :],
                        start=False, stop=True, skip_group_check=True,
                    )
                negmax = p4.tile([S, 4], f32, name="negmax")
                nc.vector.tensor_reduce(
                    out=negmax[:], in_=ps_sc[:], axis=AX.X, op=ALU.max, negate=True
                )
                exp_sb = p4.tile([S, 4, 80], f32, name="exp_sb")
                sums = p4.tile([S, 4], f32, name="sums")
                for b in range(4):
                    nc.scalar.activation(
                        exp_sb[:, b, :], ps_sc[:, b, :], AF.Exp,
                        bias=negmax[:, b:b + 1], scale=1.0,
                        accum_out=sums[:, b:b + 1],
                    )
                rc = p4.tile([S, 4], f32, name="rc")
                nc.vector.reciprocal(rc[:], sums[:])
                A_sb = p4.tile([S, 4, 80], f32, name="A_sb")
                for b in range(4):
                    nc.vector.tensor_scalar(
                        out=A_sb[:, b, :], in0=exp_sb[:, b, :],
                        scalar1=rc[:, b:b + 1], scalar2=None, op0=ALU.mult,
                    )
                ps_at = p4ps.tile([80, 4, S], f32, name="ps_at")
                A_T = p4.tile([80, 4, S], f32, name="A_T")
                for b in range(4):
                    nc.tensor.transpose(ps_at[:, b, :], A_sb[:, b, :], ident[0:S, 0:S])
                nc.vector.tensor_copy(A_T[:], ps_at[:])
                yc_ps = p4ps.tile([128, 4, NLOC], f32, name="yc_ps")
                for ho in range(4):
                    hos = slice(ho * 128, (ho + 1) * 128)
                    rhsH = hsel[:, :, :, 0, :].transpose([0, 1, 3, 2])
                    rhsL = hsel[:, :, :, 1, :].transpose([0, 1, 3, 2])
                    for k in range(4):
                        nc.tensor.matmul(
                            yc_ps[:, ho, :], wc1H[:, k, hos], rhsH[:, k, :, :],
                            start=(k == 0), stop=False,
                        )
                        nc.tensor.matmul(
                            yc_ps[:, ho, :], wc1H[:, k, hos], rhsL[:, k, :, :],
                            start=False, stop=False,
                        )
                        nc.tensor.matmul(
                            yc_ps[:, ho, :], wc1L[:, k, hos], rhsH[:, k, :, :],
                            start=False, stop=False,
                        )
                    for b in range(4):
                        nc.tensor.matmul(
                            yc_ps[:, ho, b * S:(b + 1) * S], po2[:, b, hos], A_T[:, b, :],
                            start=False, stop=(b == 3), skip_group_check=True,
                        )
                ycsb = p4.tile([128, 4, NLOC], f32, name="ycsb")
                for ho in range(4):
                    nc.scalar.activation(
                        ycsb[:, ho, :], yc_ps[:, ho, :], AF.Tanh,
                        bias=cb_sb[:, ho:ho + 1], scale=1.0,
                    )
                ycH = p4.tile([128, 4, NLOC], f16, name="ycH")
                ycL = p4.tile([128, 4, NLOC], f16, name="ycL")
                nc.vector.tensor_copy(ycH[:], ycsb[:])
                nc.vector.tensor_tensor(out=ycL[:], in0=ycsb[:], in1=ycH[:], op=ALU.subtract)
                nc.sync.dma_start(rkp(yct_shard[0], "(k p) s -> p k s"), ycH[:])
                nc.sync.dma_start(rkp(yct_shard[1], "(k p) s -> p k s"), ycL[:])
            nc.gpsimd.collective_compute(
                "AllGather", ALU.bypass, replica_groups=[list(range(NC))],
                ins=[yct_shard.opt()], outs=[yct_full.opt()],
            )
            mid.release()
            nc.leave_named_scope("P4_attn", sc4[0], False)
            sc5 = nc.enter_named_scope("P5_proj", False)

            # ---------- P5: YCT assembly + projection + top1 ----------
            with tc.tile_pool(name="p5", bufs=1) as p5, tc.tile_pool(
                name="p5ps", bufs=1, space="PSUM"
            ) as p5ps:
                YH = p5.tile([128, 4, 1280], f16, name="YH")
                YL = p5.tile([128, 4, 1280], f16, name="YL")
                nc.vector.memset(YH[:, :, 1248:1280], 0.0)
                nc.vector.memset(YL[:, :, 1248:1280], 0.0)
                nc.vector.tensor_copy(YH[:, :, 1248], sosH[:])
                nc.vector.tensor_copy(YL[:, :, 1248], sosL[:])
                for c in range(NC):
                    csl = slice(c * NLOC, (c + 1) * NLOC)
                    nc.sync.dma_start(
                        YH[:, :, csl],
                        yct_full[c, 0].rearrange("(k p) s -> p k s", p=128),
                    )
                    nc.sync.dma_start(
                        YL[:, :, csl],
                        yct_full[c, 1].rearrange("(k p) s -> p k s", p=128),
                    )
                YHf = YH
                YLf = YL
                mx_sb = p5.tile([128, 10], f32, name="mx_sb")
                mi_sb = p5.tile([128, 10], u32, name="mi_sb")
                nvch = (VS + 511) // 512
                VCH = [(i * 512, min(512, VS - i * 512)) for i in range(nvch)]
                for bt in range(10):
                    bsl = slice(bt * 128, (bt + 1) * 128)
                    nrow = 128 if bt < 9 else 97  # rows 1249.. are padding
                    lg = p5.tile([128, VS], f32, name=f"lg{bt}", tag="lg", bufs=2)
                    for v0, vn in VCH:
                        ps = p5ps.tile([128, 512], f32, name=f"pr{bt}_{v0}", tag="pr", bufs=4)
                        for k in range(4):
                            nc.tensor.matmul(
                                ps[:, 0:vn], YHf[:, k, bsl], woutH[:, k, v0:v0 + vn],
                                start=(k == 0), stop=False,
                            )
                        for k in range(4):
                            nc.tensor.matmul(
                                ps[:, 0:vn], YHf[:, k, bsl], woutL[:, k, v0:v0 + vn],
                                start=False, stop=False,
                            )
                        for k in range(4):
                            nc.tensor.matmul(
                                ps[:, 0:vn], YLf[:, k, bsl], woutH[:, k, v0:v0 + vn],
                                start=False, stop=(k == 3),
                            )
                        nc.scalar.copy(lg[:, v0:v0 + vn], ps[:, 0:vn])
                    mx8 = p5.tile([128, 8], f32, name=f"mx8_{bt}", tag="mx8", bufs=2)
                    nc.vector.max(mx8[:], lg[:])
                    mi8 = p5.tile([128, 8], u32, name=f"mi8_{bt}", tag="mi8", bufs=2)
                    nc.vector.max_index(mi8[:], mx8[:], lg[:])
                    nc.vector.tensor_copy(mx_sb[:, bt:bt + 1], mx8[:, 0:1])
                    nc.vector.tensor_copy(mi_sb[:, bt:bt + 1], mi8[:, 0:1])
                    nc.sync.dma_start(dec[bt * 128:bt * 128 + nrow, :], lg[0:nrow, :])
                nc.sync.dma_start(mxo, mx_sb[:])
                nc.sync.dma_start(mio, mi_sb[:])
            nc.leave_named_scope("P5_proj", sc5[0], False)

    nc.compile()
    return nc


def _prep_inputs(inputs):
    output = np.asarray(inputs["output"], np.float32)
    h0 = np.asarray(inputs["h0"], np.float32)
    c0 = np.asarray(inputs["c0"], np.float32)
    embed_w = np.asarray(inputs["embed_w"], np.float32)
    W_ih = np.asarray(inputs["W_ih"], np.float32)
    W_hh = np.asarray(inputs["W_hh"], np.float32)
    b_ih = np.asarray(inputs["b_ih"], np.float32)
    b_hh = np.asarray(inputs["b_hh"], np.float32)
    attn_w = np.asarray(inputs["attn_w"], np.float32)
    attn_b = np.asarray(inputs["attn_b"], np.float32)
    concat_w = np.asarray(inputs["concat_w"], np.float32)
    concat_b = np.asarray(inputs["concat_b"], np.float32)
    Wout = np.asarray(inputs["Wout"], np.float32)
    tgt = np.asarray(inputs["target_outputs"])

    perm = np.concatenate(
        [np.arange(0, 1024), np.arange(1536, 2048), np.arange(1024, 1536)]
    )
    whh_hi, whh_lo = _split16(np.ascontiguousarray(W_hh[perm].T))
    wih_hi, wih_lo = _split16(np.ascontiguousarray(W_ih[perm].T))
    bsum_r = (b_ih + b_hh)[perm].astype(np.float32)
    attn_hi, attn_lo = _split16(np.concatenate([attn_w, attn_b[:, None]], axis=1))
    wc1t_hi, wc1t_lo = _split16(np.ascontiguousarray(concat_w[:, :H].T))
    wc2t_hi, wc2t_lo = _split16(np.ascontiguousarray(concat_w[:, H:].T))

    h0t = np.ascontiguousarray(h0.T.reshape(4, 128, B).transpose(1, 0, 2))
    c0t = np.ascontiguousarray(c0.T.reshape(4, 128, B).transpose(1, 0, 2))
    sos_hi, sos_lo = _split16(np.ascontiguousarray(embed_w[SOS].reshape(4, 128).T))

    common = {
        "whh_hi": whh_hi, "whh_lo": whh_lo,
        "wih_hi": wih_hi, "wih_lo": wih_lo,
        "attn_hi": attn_hi, "attn_lo": attn_lo,
        "wc1t_hi": wc1t_hi, "wc1t_lo": wc1t_lo,
        "wc2t_hi": wc2t_hi, "wc2t_lo": wc2t_lo,
        "h0t": h0t, "c0t": c0t, "sos_hi": sos_hi, "sos_lo": sos_lo,
        "bsum": bsum_r[None, :],
        "cbias": np.ascontiguousarray(concat_b.reshape(4, 128).T),
    }

    in_maps = []
    tgti = tgt.astype(np.int64)
    for c in range(NC):
        m = dict(common)
        wh, wl = _split16(np.ascontiguousarray(Wout[c * VS:(c + 1) * VS].T))
        m["wout_hi"], m["wout_lo"] = wh, wl
        bsl = slice(4 * c, 4 * c + 4)
        xloc = embed_w[tgti[bsl, 0:S]]  # [4, 39, 512]
        xh, xl = _split16(
            np.ascontiguousarray(xloc.transpose(2, 0, 1).reshape(512, NLOC))
        )
        m["xt_hi"] = np.ascontiguousarray(xh.reshape(4, 128, NLOC).transpose(1, 0, 2))
        m["xt_lo"] = np.ascontiguousarray(xl.reshape(4, 128, NLOC).transpose(1, 0, 2))
        m["out_loc"] = np.ascontiguousarray(output[bsl].reshape(BLOC * L, H))
        r = np.zeros((128, NC), np.float32)
        r[:, c] = 1.0
        m["rank_oh"] = r
        in_maps.append(m)
    return in_maps


def run(inputs, trace=False):
    from concourse.bass_utils import run_bass_kernel_spmd

    if "prog" not in _CACHE:
        _CACHE["prog"] = _build_program()
    nc = _CACHE["prog"]
    in_maps = _prep_inputs(inputs)
    kw = dict(trace=True, trace_cores=[0]) if trace else {}
    res = run_bass_kernel_spmd(nc, in_maps, list(range(NC)), **kw)

    dec_full = np.empty((B, T, V), np.float32)
    for c in range(NC):
        dc = res.results[c]["dec"]
        vs = slice(c * VS, (c + 1) * VS)
        dec_full[:, 1:, vs] = dc[0:B * S].reshape(B, S, VS)
        dec_full[:, 0, vs] = dc[B * S]
    bout = np.asarray(inputs["bout"], np.float32)
    if bout.any():
        dec_full = dec_full + bout[None, None, :]
    mx = np.stack([res.results[c]["mxo"].T.reshape(-1) for c in range(NC)])
    mi = np.stack([res.results[c]["mio"].T.reshape(-1) for c in range(NC)])
    mx = mx[:, 0:B * S]
    mi = mi[:, 0:B * S]
    win = np.argmax(mx, axis=0)
    symc = mi[win, np.arange(B * S)].astype(np.int64) + win * VS
    sym = np.empty((B, T), np.int64)
    sym[:, 1:] = symc.reshape(B, S)
    sym[:, 0] = SOS
    sym = sym.astype(np.asarray(inputs["target_outputs"]).dtype)
    return dec_full, sym, res


def kernel(**inputs):
    dec, sym, _ = run(inputs, trace=False)
    return dec, sym


# revision 14
# speedup vs baseline: 1.0445x; 1.0445x over previous
"""Trainium2 Bass kernel for nn_Decoder (LSTM + dot-attention + vocab projection).

Distribution (8 NeuronCores, single SPMD launch):
- fp16 hi/lo x3-term matmul splits everywhere -> fp32-identical numerics at
  1 cyc/row on the PE (plain fp32 is 4 cyc/row; plain bf16/f32r flip argmaxes).
- P1: XG = x @ W_ih.T (+gate biases), batch-sharded (4 rows/core) + AllGather.
- P2: po = output @ attn_w, po2 = output @ Wc2.T for the core's 4 batch rows.
- P3: LSTM recurrence replicated on all cores (teacher forcing makes attention
  independent of the recurrence), transposed layout [H-part, batch-free].
- P4: attention + yc for local batch rows, batched over all 39 steps; second
  AllGather shares yc.
- P5: vocab projection V-sharded (3750 rows/core) with on-device top-1.
Host does input marshalling (embedding gather, transposes, fp16 splits,
shards) and output unsharding (concat + 8-way argmax combine).
"""

import numpy as np

B, L, T, H, V = 32, 80, 40, 512, 30000
SOS = 1
S = T - 1       # 39 decode steps needed
NC = 8
VS = V // NC    # 3750
BLOC = B // NC  # 4
BT = B * T      # 1280 projection rows (index b*40+s; s=0 rows are SOS)
NLOC = BLOC * S  # 156

_CACHE = {}


def _split16(x):
    hi = x.astype(np.float16)
    lo = (x.astype(np.float32) - hi.astype(np.float32)).astype(np.float16)
    return hi, lo


def _build_program():
    import concourse.bass as bass
    import concourse.mybir as mybir
    import concourse.tile as tile
    from concourse import bacc
    from concourse.masks import make_identity

    f32 = mybir.dt.float32
    f16 = mybir.dt.float16
    u32 = mybir.dt.uint32
    AF = mybir.ActivationFunctionType
    ALU = mybir.AluOpType
    AX = mybir.AxisListType

    nc = bacc.Bacc("TRN2", target_bir_lowering=False, debug=False, num_devices=NC)

    def din(name, shape, dt=f32):
        return nc.dram_tensor(name, shape, dt, kind="ExternalInput").ap()

    def dout(name, shape, dt=f32):
        return nc.dram_tensor(name, shape, dt, kind="ExternalOutput").ap()

    whh_hi = din("whh_hi", [512, 2048], f16)
    whh_lo = din("whh_lo", [512, 2048], f16)
    wih_hi = din("wih_hi", [512, 2048], f16)
    wih_lo = din("wih_lo", [512, 2048], f16)
    attn_hi = din("attn_hi", [512, 513], f16)   # attn_w | attn_b col
    attn_lo = din("attn_lo", [512, 513], f16)
    wc1t_hi = din("wc1t_hi", [512, 512], f16)
    wc1t_lo = din("wc1t_lo", [512, 512], f16)
    wc2t_hi = din("wc2t_hi", [512, 512], f16)
    wc2t_lo = din("wc2t_lo", [512, 512], f16)
    wout_hi = din("wout_hi", [512, VS], f16)    # per-core V-shard, transposed
    wout_lo = din("wout_lo", [512, VS], f16)
    xt_hi = din("xt_hi", [128, 4, NLOC], f16)   # per-core embed(x).T slice
    xt_lo = din("xt_lo", [128, 4, NLOC], f16)
    sos_hi = din("sos_hi", [128, 4], f16)
    sos_lo = din("sos_lo", [128, 4], f16)
    out_loc = din("out_loc", [BLOC * L, 512])
    h0t = din("h0t", [128, 4, 32])
    c0t = din("c0t", [128, 4, 32])
    bsum = din("bsum", [1, 2048])
    cbias = din("cbias", [128, 4])
    rank_oh = din("rank_oh", [128, NC])

    dec = dout("dec", [1249, VS])
    mxo = dout("mxo", [128, 10])
    mio = dout("mio", [128, 10], u32)

    def rkp(ap, expr="(k p) x -> p k x"):
        return ap.rearrange(expr, p=128)

    with tile.TileContext(nc) as tc:
        with tc.tile_pool(name="per", bufs=1) as per, tc.tile_pool(
            name="dramp", bufs=1, space="DRAM"
        ) as dramp:
            # ---------- persistent loads (live through P5) ----------
            woutH = per.tile([128, 4, VS], f16, name="woutH")
            nc.sync.dma_start(woutH, rkp(wout_hi))
            woutL = per.tile([128, 4, VS], f16, name="woutL")
            nc.sync.dma_start(woutL, rkp(wout_lo))
            sosH = per.tile([128, 4], f16, name="sosH")
            nc.sync.dma_start(sosH, sos_hi)
            sosL = per.tile([128, 4], f16, name="sosL")
            nc.sync.dma_start(sosL, sos_lo)

            # xg layouts: shard [s, p, g*b] so per-step reads are contiguous
            xg_shard = dramp.tile([S, 128, 64], f32, name="xg_shard")
            xg_full = dramp.tile(
                [NC, S, 128, 64], f32, name="xg_full", addr_space="Shared"
            )
            yct_shard = dramp.tile([2, 512, NLOC], f16, name="yct_shard")
            yct_full = dramp.tile(
                [NC, 2, 512, NLOC], f16, name="yct_full", addr_space="Shared"
            )

            mid = tc.alloc_tile_pool(name="mid", bufs=1)
            whhH = mid.tile([128, 4, 2048], f16, name="whhH")
            nc.sync.dma_start(whhH, rkp(whh_hi))
            whhL = mid.tile([128, 4, 2048], f16, name="whhL")
            nc.sync.dma_start(whhL, rkp(whh_lo))
            attnH = mid.tile([128, 4, 513], f16, name="attnH")
            nc.sync.dma_start(attnH, rkp(attn_hi))
            attnL = mid.tile([128, 4, 513], f16, name="attnL")
            nc.sync.dma_start(attnL, rkp(attn_lo))
            wc1H = mid.tile([128, 4, 512], f16, name="wc1H")
            nc.sync.dma_start(wc1H, rkp(wc1t_hi))
            wc1L = mid.tile([128, 4, 512], f16, name="wc1L")
            nc.sync.dma_start(wc1L, rkp(wc1t_lo))
            wc2H = mid.tile([128, 4, 512], f16, name="wc2H")
            nc.sync.dma_start(wc2H, rkp(wc2t_hi))
            wc2L = mid.tile([128, 4, 512], f16, name="wc2L")
            nc.sync.dma_start(wc2L, rkp(wc2t_lo))
            cb_sb = mid.tile([128, 4], f32, name="cb_sb")
            nc.sync.dma_start(cb_sb, cbias)
            rank_sb = mid.tile([128, NC], f32, name="rank_sb")
            nc.sync.dma_start(rank_sb, rank_oh)
            ones39 = mid.tile([1, S], f32, name="ones39")
            nc.vector.memset(ones39, 1.0)
            ident = mid.tile([128, 128], f32, name="ident")
            make_identity(nc, ident[:])

            # state: HT16[p, k, th, 0:32]=h_hi, [32:64]=h_lo; slot 39 holds h0
            HT16 = mid.tile([128, 4, 40, 64], f16, name="HT16")
            cT = mid.tile([128, 4, 32], f32, name="cT")
            nc.sync.dma_start(cT, c0t)
            h32i = mid.tile([128, 4, 32], f32, name="h32i")
            nc.sync.dma_start(h32i, h0t)
            nc.vector.tensor_copy(HT16[:, :, 39, 0:32], h32i[:])
            nc.vector.tensor_tensor(
                out=HT16[:, :, 39, 32:64], in0=h32i[:], in1=HT16[:, :, 39, 0:32],
                op=ALU.subtract,
            )

            # ---------- P1: XG shard + AllGather ----------
            sc1 = nc.enter_named_scope("P1_xg", False)
            with tc.tile_pool(name="p1", bufs=1) as p1, tc.tile_pool(
                name="p1ps", bufs=1, space="PSUM"
            ) as p1ps:
                wihH = p1.tile([128, 4, 2048], f16, name="wihH")
                nc.sync.dma_start(wihH, rkp(wih_hi))
                wihL = p1.tile([128, 4, 2048], f16, name="wihL")
                nc.sync.dma_start(wihL, rkp(wih_lo))
                xtH = p1.tile([128, 4, NLOC], f16, name="xtH")
                nc.sync.dma_start(xtH, xt_hi)
                xtL = p1.tile([128, 4, NLOC], f16, name="xtL")
                nc.sync.dma_start(xtL, xt_lo)
                bsum_sb = p1.tile([1, 2048], f32, name="bsum_sb")
                nc.sync.dma_start(bsum_sb, bsum)
                ones156 = p1.tile([1, NLOC], f32, name="ones156")
                nc.vector.memset(ones156, 1.0)
                xg_stage2 = p1.tile([128, S, 16, BLOC], f32, name="xg_stage2")
                for g in range(16):
                    ps = p1ps.tile([128, NLOC], f32, name=f"psxg{g}", tag="psxg", bufs=4)
                    gs = slice(g * 128, (g + 1) * 128)
                    nc.tensor.matmul(ps[:], bsum_sb[:, gs], ones156[:], start=True, stop=False)
                    for k in range(4):
                        nc.tensor.matmul(ps[:], wihH[:, k, gs], xtH[:, k, :], start=False, stop=False)
                    for k in range(4):
                        nc.tensor.matmul(ps[:], wihH[:, k, gs], xtL[:, k, :], start=False, stop=False)
                    for k in range(4):
                        nc.tensor.matmul(ps[:], wihL[:, k, gs], xtH[:, k, :], start=False, stop=(k == 3))
                    nc.scalar.copy(
                        xg_stage2[:, :, g, :], ps[:].rearrange("p (b s) -> p s b", b=BLOC)
                    )
                nc.sync.dma_start(
                    xg_shard.rearrange("s p gb -> p s gb"), 
                    xg_stage2[:].rearrange("p s g b -> p s (g b)"),
                )
            nc.gpsimd.collective_compute(
                "AllGather", ALU.bypass, replica_groups=[list(range(NC))],
                ins=[xg_shard.opt()], outs=[xg_full.opt()],
            )
            nc.leave_named_scope("P1_xg", sc1[0], False)
            sc2 = nc.enter_named_scope("P2_po", False)

            # ---------- P2: outT, po (+s0), po2 ----------
            poH = mid.tile([128, 4, 4, 80], f16, name="poH")
            poL = mid.tile([128, 4, 4, 80], f16, name="poL")
            s0_sb = mid.tile([1, 4, 80], f32, name="s0_sb")
            po2 = mid.tile([80, 4, 512], f32, name="po2")
            with tc.tile_pool(name="p2", bufs=1) as p2, tc.tile_pool(
                name="p2ps", bufs=1, space="PSUM"
            ) as p2ps:
                oin = p2.tile([128, 3, 512], f32, name="oin")
                for rt, rsz in ((0, 128), (1, 128), (2, 64)):
                    nc.sync.dma_start(
                        oin[0:rsz, rt, :], out_loc[rt * 128:rt * 128 + rsz, :]
                    )
                ot32 = p2.tile([128, 4, 320], f32, name="ot32")
                for rt, rsz in ((0, 128), (1, 128), (2, 64)):
                    for hk in range(4):
                        pst = p2ps.tile([128, 128], f32, name=f"pst{rt}_{hk}", tag="pst", bufs=2)
                        nc.tensor.transpose(
                            pst[:, 0:rsz],
                            oin[0:rsz, rt, hk * 128:(hk + 1) * 128],
                            ident[0:rsz, 0:rsz],
                        )
                        nc.scalar.copy(ot32[:, hk, rt * 128:rt * 128 + rsz], pst[:, 0:rsz])
                otH = p2.tile([128, 4, 320], f16, name="otH")
                otL = p2.tile([128, 4, 320], f16, name="otL")
                nc.vector.tensor_copy(otH[:], ot32[:])
                nc.vector.tensor_tensor(out=otL[:], in0=ot32[:], in1=otH[:], op=ALU.subtract)
                for m in range(5):  # 4 kout tiles + s0 row
                    msl = slice(m * 128, (m + 1) * 128) if m < 4 else slice(512, 513)
                    mp = 128 if m < 4 else 1
                    ps = p2ps.tile([128, 320], f32, name=f"pspo{m}", tag="pspo", bufs=2)
                    for k in range(4):
                        nc.tensor.matmul(ps[0:mp, :], attnH[:, k, msl], otH[:, k, :], start=(k == 0), stop=False)
                    for k in range(4):
                        nc.tensor.matmul(ps[0:mp, :], attnH[:, k, msl], otL[:, k, :], start=False, stop=False)
                    for k in range(4):
                        nc.tensor.matmul(ps[0:mp, :], attnL[:, k, msl], otH[:, k, :], start=False, stop=(k == 3))
                    if m < 4:
                        nc.vector.tensor_copy(
                            poH[:, m, :, :], ps[:].rearrange("p (b l) -> p b l", b=4)
                        )
                        nc.vector.tensor_tensor(
                            out=poL[:, m, :, :],
                            in0=ps[:].rearrange("p (b l) -> p b l", b=4),
                            in1=poH[:, m, :, :], op=ALU.subtract,
                        )
                    else:
                        nc.vector.tensor_copy(
                            s0_sb[:], ps[0:1, :].rearrange("p (b l) -> p b l", b=4)
                        )
                for b in range(4):
                    ps = p2ps.tile([80, 512], f32, name=f"pspo2_{b}", tag="pspo2", bufs=2)
                    bsl = slice(b * 80, b * 80 + 80)
                    for k in range(4):
                        nc.tensor.matmul(ps[:], otH[:, k, bsl], wc2H[:, k, :], start=(k == 0), stop=False)
                    for k in range(4):
                        nc.tensor.matmul(ps[:], otL[:, k, bsl], wc2H[:, k, :], start=False, stop=False)
                    for k in range(4):
                        nc.tensor.matmul(ps[:], otH[:, k, bsl], wc2L[:, k, :], start=False, stop=(k == 3))
                    nc.scalar.copy(po2[:, b, :], ps[:])

            nc.leave_named_scope("P2_po", sc2[0], False)
            sc3 = nc.enter_named_scope("P3_rec", False)

            # ---------- P3: LSTM recurrence ----------
            def xg_step_src(t):
                # [p][c][g*b] with 256B-contiguous per-partition runs
                return bass.AP(
                    tensor=xg_full.tensor,
                    offset=xg_full.offset + t * 128 * 64,
                    ap=[[64, 128], [S * 128 * 64, NC], [1, 64]],
                )

            with tc.tile_pool(name="p3", bufs=1) as p3, tc.tile_pool(
                name="p3ps", bufs=1, space="PSUM"
            ) as p3ps:
                for t in range(S):
                    xgt = p3.tile([128, NC, 16, BLOC], f32, name=f"xgt{t}", tag="xgt", bufs=3)
                    nc.sync.dma_start(xgt[:], xg_step_src(t))
                    # per-half psum: [p, grp(i,f,o,g~), hc(2), {hi,lo}, b] (1 bank)
                    psgh = [
                        p3ps.tile([128, 4, 2, 2, 32], f32, name=f"psg{t}_{hh}", tag=f"psg{hh}", bufs=2)
                        for hh in range(2)
                    ]
                    th = 39 if t == 0 else t - 1
                    xgt2 = p3.tile([128, 16, NC, BLOC], f32, name=f"xgt2_{t}", tag="xgt2", bufs=2)
                    nc.vector.tensor_copy(xgt2[:], xgt[:].transpose([0, 2, 1, 3]))
                    xgv = xgt2[:].rearrange("p (grp hc) c b -> p grp hc c b", hc=4)
                    for half in range(2):
                        hsl = slice(2 * half, 2 * half + 2)
                        psg = psgh[half]
                        # PE: all gtiles of this half; k-pairs split so next
                        # step can start on half-produced h chunks
                        for grp in range(4):
                            for hc in (2 * half, 2 * half + 1):
                                g = grp * 4 + hc
                                gs = slice(g * 128, (g + 1) * 128)
                                out_w = psg[:, grp, hc - 2 * half, :, :]
                                out_h = psg[:, grp, hc - 2 * half, 0, :]
                                for kk in range(2):
                                    for k in (2 * kk, 2 * kk + 1):
                                        nc.tensor.matmul(
                                            out_w, whhH[:, k, gs], HT16[:, k, th, :],
                                            start=(k == 0), stop=False,
                                        )
                                    for k in (2 * kk, 2 * kk + 1):
                                        nc.tensor.matmul(
                                            out_h, whhL[:, k, gs], HT16[:, k, th, 0:32],
                                            start=False, stop=(k == 3),
                                            skip_group_check=True,
                                        )
                        gsb = p3.tile([128, 4, 2, 32], f32, name=f"gsb{t}_{half}", tag=f"gsb{half}", bufs=2)
                        nc.vector.tensor_tensor(
                            out=gsb[:].rearrange("p grp hc (c b) -> p grp hc c b", c=NC),
                            in0=xgv[:, :, hsl, :, :],
                            in1=psg[:, :, :, 0, :].rearrange("p grp hc (c b) -> p grp hc c b", c=NC),
                            op=ALU.add,
                        )
                        nc.vector.tensor_tensor(
                            out=gsb[:], in0=gsb[:], in1=psg[:, :, :, 1, :], op=ALU.add,
                        )
                        asb = p3.tile([128, 3, 2, 32], f32, name=f"asb{t}_{half}", tag=f"asb{half}", bufs=2)
                        nc.scalar.activation(asb[:], gsb[:, 0:3, :, :], AF.Sigmoid)
                        tnh = p3.tile([128, 2, 32], f32, name=f"tnh{t}_{half}", tag=f"tnh{half}", bufs=2)
                        nc.scalar.activation(tnh[:], gsb[:, 3, :, :], AF.Tanh)
                        t1 = p3.tile([128, 2, 32], f32, name=f"t1_{t}_{half}", tag=f"t1{half}", bufs=2)
                        nc.vector.tensor_tensor(out=t1[:], in0=asb[:, 0, :, :], in1=tnh[:], op=ALU.mult)
                        t2 = p3.tile([128, 2, 32], f32, name=f"t2_{t}_{half}", tag=f"t2{half}", bufs=2)
                        nc.vector.tensor_tensor(out=t2[:], in0=asb[:, 1, :, :], in1=cT[:, hsl, :], op=ALU.mult)
                        nc.vector.tensor_tensor(out=cT[:, hsl, :], in0=t1[:], in1=t2[:], op=ALU.add)
                        tc32 = p3.tile([128, 2, 32], f32, name=f"tc32_{t}_{half}", tag=f"tc32{half}", bufs=2)
                        nc.scalar.activation(tc32[:], cT[:, hsl, :], AF.Tanh)
                        h32 = p3.tile([128, 2, 32], f32, name=f"h32_{t}_{half}", tag=f"h32{half}", bufs=2)
                        nc.vector.tensor_tensor(out=h32[:], in0=asb[:, 2, :, :], in1=tc32[:], op=ALU.mult)
                        nc.scalar.copy(HT16[:, hsl, t, 0:32], h32[:])
                        nc.vector.tensor_tensor(
                            out=HT16[:, hsl, t, 32:64], in0=h32[:],
                            in1=HT16[:, hsl, t, 0:32], op=ALU.subtract,
                        )
            nc.leave_named_scope("P3_rec", sc3[0], False)
            sc4 = nc.enter_named_scope("P4_attn", False)

            # ---------- P4: attention + yc (local 4 b) ----------
            with tc.tile_pool(name="p4", bufs=1) as p4, tc.tile_pool(
                name="p4ps", bufs=1, space="PSUM"
            ) as p4ps:
                # local-h extraction via rank one-hot: hsel32 = sum_c r[c]*HT16[..4c+b..]
                hsel32 = p4.tile([128, 4, S, 2, 4], f32, name="hsel32")
                tmp = p4.tile([128, 4, S, 2, 4], f32, name="tmpsel")
                for c in range(NC):
                    dst = tmp if c > 0 else hsel32
                    nc.vector.tensor_scalar(
                        out=dst[:, :, :, 0, :], in0=HT16[:, :, 0:S, 4 * c:4 * c + 4],
                        scalar1=rank_sb[:, c:c + 1], scalar2=None, op0=ALU.mult,
                    )
                    nc.vector.tensor_scalar(
                        out=dst[:, :, :, 1, :],
                        in0=HT16[:, :, 0:S, 32 + 4 * c:32 + 4 * c + 4],
                        scalar1=rank_sb[:, c:c + 1], scalar2=None, op0=ALU.mult,
                    )
                    if c > 0:
                        nc.vector.tensor_tensor(out=hsel32[:], in0=hsel32[:], in1=tmp[:], op=ALU.add)
                hsel = p4.tile([128, 4, S, 2, 4], f16, name="hsel")
                nc.vector.tensor_copy(hsel[:], hsel32[:])

                ps_sc = p4ps.tile([S, 4, 80], f32, name="ps_sc")
                for b in range(4):
                    for k in range(4):
                        nc.tensor.matmul(
                            ps_sc[:, b, :], hsel[:, k, :, 0, b], poH[:, k, b, :],
                            start=(k == 0), stop=False,
                        )
                    for k in range(4):
                        nc.tensor.matmul(
                            ps_sc[:, b, :], hsel[:, k, :, 1, b], poH[:, k, b, :],
                            start=False, stop=False,
                        )
                    for k in range(4):
                        nc.tensor.matmul(
                            ps_sc[:, b, :], hsel[:, k, :, 0, b], poL[:, k, b, :],
                            start=False, stop=False,
                        )
                    nc.tensor.matmul(
                        ps_sc[:, b, :], ones39[:], s0_sb[:, b, :],
                        start=False, stop=True, skip_group_check=True,
                    )
                negmax = p4.tile([S, 4], f32, name="negmax")
                nc.vector.tensor_reduce(
                    out=negmax[:], in_=ps_sc[:], axis=AX.X, op=ALU.max, negate=True
                )
                exp_sb = p4.tile([S, 4, 80], f32, name="exp_sb")
                sums = p4.tile([S, 4], f32, name="sums")
                for b in range(4):
                    nc.scalar.activation(
                        exp_sb[:, b, :], ps_sc[:, b, :], AF.Exp,
                        bias=negmax[:, b:b + 1], scale=1.0,
                        accum_out=sums[:, b:b + 1],
                    )
                rc = p4.tile([S, 4], f32, name="rc")
                nc.vector.reciprocal(rc[:], sums[:])
                A_sb = p4.tile([S, 4, 80], f32, name="A_sb")
                for b in range(4):
                    nc.vector.tensor_scalar(
                        out=A_sb[:, b, :], in0=exp_sb[:, b, :],
                        scalar1=rc[:, b:b + 1], scalar2=None, op0=ALU.mult,
                    )
                ps_at = p4ps.tile([80, 4, S], f32, name="ps_at")
                A_T = p4.tile([80, 4, S], f32, name="A_T")
                for b in range(4):
                    nc.tensor.transpose(ps_at[:, b, :], A_sb[:, b, :], ident[0:S, 0:S])
                nc.vector.tensor_copy(A_T[:], ps_at[:])
                yc_ps = p4ps.tile([128, 4, NLOC], f32, name="yc_ps")
                for ho in range(4):
                    hos = slice(ho * 128, (ho + 1) * 128)
                    rhsH = hsel[:, :, :, 0, :].transpose([0, 1, 3, 2])
                    rhsL = hsel[:, :, :, 1, :].transpose([0, 1, 3, 2])
                    for k in range(4):
                        nc.tensor.matmul(
                            yc_ps[:, ho, :], wc1H[:, k, hos], rhsH[:, k, :, :],
                            start=(k == 0), stop=False,
                        )
                        nc.tensor.matmul(
                            yc_ps[:, ho, :], wc1H[:, k, hos], rhsL[:, k, :, :],
                            start=False, stop=False,
                        )
                        nc.tensor.matmul(
                            yc_ps[:, ho, :], wc1L[:, k, hos], rhsH[:, k, :, :],
                            start=False, stop=False,
                        )
                    for b in range(4):
                        nc.tensor.matmul(
                            yc_ps[:, ho, b * S:(b + 1) * S], po2[:, b, hos], A_T[:, b, :],
                            start=False, stop=(b == 3), skip_group_check=True,
                        )
                ycsb = p4.tile([128, 4, NLOC], f32, name="ycsb")
                for ho in range(4):
                    nc.scalar.activation(
                        ycsb[:, ho, :], yc_ps[:, ho, :], AF.Tanh,
                        bias=cb_sb[:, ho:ho + 1], scale=1.0,
                    )
                ycH = p4.tile([128, 4, NLOC], f16, name="ycH")
                ycL = p4.tile([128, 4, NLOC], f16, name="ycL")
                nc.vector.tensor_copy(ycH[:], ycsb[:])
                nc.vector.tensor_tensor(out=ycL[:], in0=ycsb[:], in1=ycH[:], op=ALU.subtract)
                nc.sync.dma_start(rkp(yct_shard[0], "(k p) s -> p k s"), ycH[:])
                nc.sync.dma_start(rkp(yct_shard[1], "(k p) s -> p k s"), ycL[:])
            nc.gpsimd.collective_compute(
                "AllGather", ALU.bypass, replica_groups=[list(range(NC))],
                ins=[yct_shard.opt()], outs=[yct_full.opt()],
            )
            mid.release()
            nc.leave_named_scope("P4_attn", sc4[0], False)
            sc5 = nc.enter_named_scope("P5_proj", False)

            # ---------- P5: YCT assembly + projection + top1 ----------
            with tc.tile_pool(name="p5", bufs=1) as p5, tc.tile_pool(
                name="p5ps", bufs=1, space="PSUM"
            ) as p5ps:
                YH = p5.tile([128, 4, 1280], f16, name="YH")
                YL = p5.tile([128, 4, 1280], f16, name="YL")
                nc.vector.memset(YH[:, :, 1248:1280], 0.0)
                nc.vector.memset(YL[:, :, 1248:1280], 0.0)
                nc.vector.tensor_copy(YH[:, :, 1248], sosH[:])
                nc.vector.tensor_copy(YL[:, :, 1248], sosL[:])
                for c in range(NC):
                    csl = slice(c * NLOC, (c + 1) * NLOC)
                    nc.sync.dma_start(
                        YH[:, :, csl],
                        yct_full[c, 0].rearrange("(k p) s -> p k s", p=128),
                    )
                    nc.sync.dma_start(
                        YL[:, :, csl],
                        yct_full[c, 1].rearrange("(k p) s -> p k s", p=128),
                    )
                YHf = YH
                YLf = YL
                mx_sb = p5.tile([128, 10], f32, name="mx_sb")
                mi_sb = p5.tile([128, 10], u32, name="mi_sb")
                nvch = (VS + 511) // 512
                VCH = [(i * 512, min(512, VS - i * 512)) for i in range(nvch)]
                for bt in range(10):
                    bsl = slice(bt * 128, (bt + 1) * 128)
                    nrow = 128 if bt < 9 else 97  # rows 1249.. are padding
                    lg = p5.tile([128, VS], f32, name=f"lg{bt}", tag="lg", bufs=2)
                    for v0, vn in VCH:
                        ps = p5ps.tile([128, 512], f32, name=f"pr{bt}_{v0}", tag="pr", bufs=4)
                        for k in range(4):
                            nc.tensor.matmul(
                                ps[:, 0:vn], YHf[:, k, bsl], woutH[:, k, v0:v0 + vn],
                                start=(k == 0), stop=False,
                            )
                        for k in range(4):
                            nc.tensor.matmul(
                                ps[:, 0:vn], YHf[:, k, bsl], woutL[:, k, v0:v0 + vn],
                                start=False, stop=False,
                            )
                        for k in range(4):
                            nc.tensor.matmul(
                                ps[:, 0:vn], YLf[:, k, bsl], woutH[:, k, v0:v0 + vn],
                                start=False, stop=(k == 3),
                            )
                        nc.scalar.copy(lg[:, v0:v0 + vn], ps[:, 0:vn])
                    mx8 = p5.tile([128, 8], f32, name=f"mx8_{bt}", tag="mx8", bufs=2)
                    nc.vector.max(mx8[:], lg[:])
                    mi8 = p5.tile([128, 8], u32, name=f"mi8_{bt}", tag="mi8", bufs=2)
                    nc.vector.max_index(mi8[:], mx8[:], lg[:])
                    nc.vector.tensor_copy(mx_sb[:, bt:bt + 1], mx8[:, 0:1])
                    nc.vector.tensor_copy(mi_sb[:, bt:bt + 1], mi8[:, 0:1])
                    nc.sync.dma_start(dec[bt * 128:bt * 128 + nrow, :], lg[0:nrow, :])
                nc.sync.dma_start(mxo, mx_sb[:])
                nc.sync.dma_start(mio, mi_sb[:])
            nc.leave_named_scope("P5_proj", sc5[0], False)

    nc.compile()
    return nc


def _prep_inputs(inputs):
    output = np.asarray(inputs["output"], np.float32)
    h0 = np.asarray(inputs["h0"], np.float32)
    c0 = np.asarray(inputs["c0"], np.float32)
    embed_w = np.asarray(inputs["embed_w"], np.float32)
    W_ih = np.asarray(inputs["W_ih"], np.float32)
    W_hh = np.asarray(inputs["W_hh"], np.float32)
    b_ih = np.asarray(inputs["b_ih"], np.float32)
    b_hh = np.asarray(inputs["b_hh"], np.float32)
    attn_w = np.asarray(inputs["attn_w"], np.float32)
    attn_b = np.asarray(inputs["attn_b"], np.float32)
    concat_w = np.asarray(inputs["concat_w"], np.float32)
    concat_b = np.asarray(inputs["concat_b"], np.float32)
    Wout = np.asarray(inputs["Wout"], np.float32)
    tgt = np.asarray(inputs["target_outputs"])

    perm = np.concatenate(
        [np.arange(0, 1024), np.arange(1536, 2048), np.arange(1024, 1536)]
    )
    whh_hi, whh_lo = _split16(np.ascontiguousarray(W_hh[perm].T))
    wih_hi, wih_lo = _split16(np.ascontiguousarray(W_ih[perm].T))
    bsum_r = (b_ih + b_hh)[perm].astype(np.float32)
    attn_hi, attn_lo = _split16(np.concatenate([attn_w, attn_b[:, None]], axis=1))
    wc1t_hi, wc1t_lo = _split16(np.ascontiguousarray(concat_w[:, :H].T))
    wc2t_hi, wc2t_lo = _split16(np.ascontiguousarray(concat_w[:, H:].T))

    h0t = np.ascontiguousarray(h0.T.reshape(4, 128, B).transpose(1, 0, 2))
    c0t = np.ascontiguousarray(c0.T.reshape(4, 128, B).transpose(1, 0, 2))
    sos_hi, sos_lo = _split16(np.ascontiguousarray(embed_w[SOS].reshape(4, 128).T))

    common = {
        "whh_hi": whh_hi, "whh_lo": whh_lo,
        "wih_hi": wih_hi, "wih_lo": wih_lo,
        "attn_hi": attn_hi, "attn_lo": attn_lo,
        "wc1t_hi": wc1t_hi, "wc1t_lo": wc1t_lo,
        "wc2t_hi": wc2t_hi, "wc2t_lo": wc2t_lo,
        "h0t": h0t, "c0t": c0t, "sos_hi": sos_hi, "sos_lo": sos_lo,
        "bsum": bsum_r[None, :],
        "cbias": np.ascontiguousarray(concat_b.reshape(4, 128).T),
    }

    in_maps = []
    tgti = tgt.astype(np.int64)
    for c in range(NC):
        m = dict(common)
        wh, wl = _split16(np.ascontiguousarray(Wout[c * VS:(c + 1) * VS].T))
        m["wout_hi"], m["wout_lo"] = wh, wl
        bsl = slice(4 * c, 4 * c + 4)
        xloc = embed_w[tgti[bsl, 0:S]]  # [4, 39, 512]
        xh, xl = _split16(
            np.ascontiguousarray(xloc.transpose(2, 0, 1).reshape(512, NLOC))
        )
        m["xt_hi"] = np.ascontiguousarray(xh.reshape(4, 128, NLOC).transpose(1, 0, 2))
        m["xt_lo"] = np.ascontiguousarray(xl.reshape(4, 128, NLOC).transpose(1, 0, 2))
        m["out_loc"] = np.ascontiguousarray(output[bsl].reshape(BLOC * L, H))
        r = np.zeros((128, NC), np.float32)
        r[:, c] = 1.0
        m["rank_oh"] = r
        in_maps.append(m)
    return in_maps


def run(inputs, trace=False):
    from concourse.bass_utils import run_bass_kernel_spmd

    if "prog" not in _CACHE:
        _CACHE["prog"] = _build_program()
    nc = _CACHE["prog"]
    in_maps = _prep_inputs(inputs)
    kw = dict(trace=True, trace_cores=[0]) if trace else {}
    res = run_bass_kernel_spmd(nc, in_maps, list(range(NC)), **kw)

    dec_full = np.empty((B, T, V), np.float32)
    for c in range(NC):
        dc = res.results[c]["dec"]
        vs = slice(c * VS, (c + 1) * VS)
        dec_full[:, 1:, vs] = dc[0:B * S].reshape(B, S, VS)
        dec_full[:, 0, vs] = dc[B * S]
    bout = np.asarray(inputs["bout"], np.float32)
    if bout.any():
        dec_full = dec_full + bout[None, None, :]
    mx = np.stack([res.results[c]["mxo"].T.reshape(-1) for c in range(NC)])
    mi = np.stack([res.results[c]["mio"].T.reshape(-1) for c in range(NC)])
    mx = mx[:, 0:B * S]
    mi = mi[:, 0:B * S]
    win = np.argmax(mx, axis=0)
    symc = mi[win, np.arange(B * S)].astype(np.int64) + win * VS
    sym = np.empty((B, T), np.int64)
    sym[:, 1:] = symc.reshape(B, S)
    sym[:, 0] = SOS
    sym = sym.astype(np.asarray(inputs["target_outputs"]).dtype)
    return dec_full, sym, res


def kernel(**inputs):
    dec, sym, _ = run(inputs, trace=False)
    return dec, sym
